# revision 21
# baseline (speedup 1.0000x reference)
"""Trainium2 Bass kernel for nn_GAT_1675037246077 (2-layer GAT + linear head).

Strategy (8 NeuronCores, SPMD single NEFF):
 - Destination-sharded: core c owns SH=12544 dst nodes; nodes assigned to cores
   by a host-side greedy that spreads each node's in-edge sources evenly across
   the 4 core-pairs ("quarters"), then within each core sorted by in-degree and
   packed into 98 tiles of 128 nodes.
 - Layer 1: host expands x into edge-slot order (xexpT); per-slot h1 = xe @ Waug1
   on the PE; scores exp(leakyrelu) with alpha_dst per-partition; weighted
   aggregation via identity-matmul PSUM accumulation; exact pad-slot z corrections.
 - Layer 2: table2[row=pos(node)] = [h2|as2] built shard-wise, AllGather'd,
   repacked into a 256B-row table with one explicit zero row per quarter.
   Edge gathering uses dma_gather (InstDMAGatherAnt): one issue per
   (tile-group, quarter) with int16 quarter-local row indices -> the gather's
   native output layout IS the slot grid. Per-quarter slot counts are uniform
   within a group (J maxed over the group's tiles); the greedy quarter balance
   keeps the padding small.
 - No segment-max: scores are bounded (|e| < 2), exp is safe.
"""
import numpy as np
import ml_dtypes

from concourse import mybir, tile, bacc
import concourse.bass as bass
from concourse import ap_utils
from concourse.bass_utils import run_bass_kernel_spmd
from concourse.masks import make_identity

P = 128
AF = mybir.ActivationFunctionType
ALU = mybir.AluOpType
BF16 = mybir.dt.bfloat16
F32 = mybir.dt.float32
I16 = mybir.dt.int16
NPBF16 = ml_dtypes.bfloat16

SELU_SCALE = 1.0507009873554805
SELU_ALPHA_SCALE = 1.7580993408473766  # scale * alpha


class Cfg:
    def __init__(self, N, E, ncores, fin=16, h1=3, c1=32, c2=32, ncout=16):
        self.N, self.E, self.ncores = N, E, ncores
        self.FIN, self.H1, self.C1, self.C2, self.NCOUT = fin, h1, c1, c2, ncout
        self.CW1 = h1 * c1              # 96
        self.F1 = self.CW1 + h1         # table1 cols (99)
        self.F2 = c2 + 1 + 3            # table2 cols (36): [h2|as2|pad3]
        self.SH = ((N + ncores - 1) // ncores + P - 1) // P * P
        self.TPC = self.SH // P
        self.NPAD = self.SH * ncores
        self.WAUG2_C = c2 + 2           # [W2(32)|as2|ad2]
        self.QS = 2 * self.SH           # quarter size (rows), int16-addressable
        self.QROWS = self.QS + 1        # + zero row
        self.TABW = 128                 # padded table row width (bf16) = 256B


def _group_plan(J, cap):
    groups = []
    t = 0
    n = len(J)
    while t < n:
        j = J[t]
        nt = 1
        while t + nt < n and J[t + nt] == j and (nt + 1) * j <= cap and nt < 6:
            nt += 1
        groups.append((t, nt, int(j)))
        t += nt
    return groups


def _group_plan2(J2, cap, max_nt=16):
    """Group consecutive tiles; per-quarter J maxed over the group.
    Returns [(t0, nt, (Jh0, Jh1))]."""
    groups = []
    t = 0
    n = len(J2)
    tilesum = J2.sum(axis=1)
    while t < n:
        nt = 1
        while t + nt < n and nt < max_nt:
            m = np.maximum.reduce(J2[t:t + nt + 1], axis=0)
            padded = (nt + 1) * int(m.sum())
            if padded > cap or padded > 1.15 * int(tilesum[t:t + nt + 1].sum()) + 6:
                break
            nt += 1
        m = np.maximum.reduce(J2[t:t + nt], axis=0)
        groups.append((t, nt, tuple(int(v) for v in m)))
        t += nt
    return groups


def _assign_quarters(src, dst, N, NPAD, QS, npasses=3):
    """Greedy (with refinement passes): assign each node to one of 4 quarters
    (capacity QS) so every dst node's in-edge srcs spread evenly; penalize
    exceeding the per-dst per-quarter target ceil(deg/4)."""
    deg = np.bincount(dst, minlength=NPAD)
    tgt = np.ceil(deg / 4).astype(np.float32)
    sdeg = np.bincount(src, minlength=NPAD).astype(np.int64)
    sstart = np.zeros(NPAD + 1, np.int64)
    np.cumsum(sdeg, out=sstart[1:])
    dst_by_src = dst[np.argsort(src, kind="stable")]
    cnt = np.zeros((NPAD, 4), np.float32)
    cap = np.full(4, QS, np.int64)
    qa = np.full(NPAD, -1, np.int8)
    rng = np.random.default_rng(0)
    nodes = rng.permutation(N)
    B = 1024
    for pas in range(npasses):
        for b0 in range(0, N, B):
            batch = nodes[b0:b0 + B]
            lens = sdeg[batch]
            tot = int(lens.sum())
            sc = np.zeros((len(batch), 4), np.float32)
            dd = seg = None
            if tot:
                ends = np.cumsum(lens)
                base = np.repeat(sstart[batch] - (ends - lens), lens)
                dd = dst_by_src[base + np.arange(tot)]
                seg = np.repeat(np.arange(len(batch)), lens)
            if pas > 0:
                old = qa[batch].astype(np.int64)
                cap += np.bincount(old, minlength=4)
                if dd is not None:
                    np.add.at(cnt, (dd, old[seg]), -1.0)
            if dd is not None:
                c = cnt[dd]
                np.add.at(sc, seg,
                          100.0 * np.maximum(0, c + 1 - tgt[dd][:, None]) + c)
            pen = np.where(cap > 0, (QS - cap).astype(np.float32) * 1e-4,
                           np.float32(np.inf))
            choice = np.argmin(sc + pen[None, :], axis=1).astype(np.int64)
            counts = np.bincount(choice, minlength=4)
            while np.any(counts > cap):
                q = int(np.argmax(counts - cap))
                movable = np.where(choice == q)[0]
                room = cap - counts
                q2 = int(np.argmax(room))
                nmove = min(int(counts[q] - cap[q]), int(room[q2]))
                choice[movable[-nmove:]] = q2
                counts = np.bincount(choice, minlength=4)
            qa[batch] = choice
            cap -= counts
            if dd is not None:
                np.add.at(cnt, (dd, choice[seg]), 1.0)
    pads = np.arange(N, NPAD)
    fill = np.repeat(np.arange(4), np.maximum(cap, 0))
    qa[pads] = fill[:len(pads)].astype(np.int8)
    return qa, cnt.astype(np.int32)


def preprocess(cfg, x, edge_index, W1, a_src1, a_dst1, W2, a_src2, a_dst2,
               cap2=160):
    N, E, NC = cfg.N, cfg.E, cfg.ncores
    SH, TPC, NPAD, QS = cfg.SH, cfg.TPC, cfg.NPAD, cfg.QS

    loops = np.arange(N, dtype=np.int64)
    src = np.concatenate([edge_index[0].astype(np.int64), loops])
    dst = np.concatenate([edge_index[1].astype(np.int64), loops])

    deg = np.bincount(dst, minlength=NPAD)

    qa, qcnt = _assign_quarters(src, dst, N, NPAD, QS)

    # per-quarter (deg, quarter-profile) sort -> aligned tiles across cores;
    # alternate ranks between the quarter's two cores
    perms = [None] * NC
    pos = np.empty(NPAD, np.int64)
    for q in range(4):
        nq = np.where(qa == q)[0]
        cq = qcnt[nq]
        order = np.lexsort((cq[:, 3], cq[:, 2], cq[:, 1], cq[:, 0], -deg[nq]))
        nq = nq[order]
        for par in range(2):
            c = 2 * q + par
            perm = nq[par::2]
            assert len(perm) == SH
            perms[c] = perm
            pos[perm] = c * SH + np.arange(SH)

    Jt_all = np.zeros((NC, TPC), np.int64)
    for c in range(NC):
        Jt_all[c] = deg[perms[c]].reshape(TPC, P).max(1)
    J = np.maximum(Jt_all.max(0), 1)
    SLOTS = int(J.sum())

    # CSR of edges by dst (node ids)
    e_order = np.argsort(dst, kind="stable")
    src_sorted = src[e_order]
    starts = np.zeros(NPAD + 1, np.int64)
    np.cumsum(deg, out=starts[1:])

    offs = np.zeros(TPC + 1, np.int64)
    np.cumsum(J, out=offs[1:])

    percore = []
    for c in range(NC):
        perm = perms[c]
        idx1 = np.full((P, SLOTS), NPAD - 1, np.int32)  # pad -> x=0 node
        npad1 = np.zeros((P, TPC), np.float32)
        for t in range(TPC):
            jt = int(J[t])
            o = int(offs[t])
            for p in range(P):
                node = perm[t * P + p]
                dg = int(deg[node])
                s0 = int(starts[node])
                idx1[p, o:o + dg] = src_sorted[s0:s0 + dg]
                npad1[p, t] = jt - dg
        percore.append(dict(idx1=idx1, npad1=npad1, own=perm.astype(np.int64)))

    # host-side L1 expansion: x rows in edge-slot order, transposed
    xpad = np.zeros((NPAD, cfg.FIN), np.float32)
    xpad[:N] = x
    xpadT_bf = np.ascontiguousarray(xpad.T).astype(NPBF16)
    for c in range(NC):
        cols = percore[c]["idx1"].T.ravel()
        percore[c]["xexpT"] = np.ascontiguousarray(xpadT_bf[:, cols])

    # ---- layer-2 quarter grid (int16 local rows, pad -> zero row QS) ----
    PADIDX = np.int16(QS)
    srcpos = pos[src]
    qq = srcpos // QS
    lq = srcpos % QS
    dstpos = pos[dst]
    okey = dstpos * 4 + qq
    eorder2 = np.argsort(okey, kind="stable")
    kcnt = np.bincount(okey, minlength=NPAD * 4)
    kstart = np.zeros(NPAD * 4 + 1, np.int64)
    np.cumsum(kcnt, out=kstart[1:])
    rank = np.arange(len(eorder2)) - kstart[okey[eorder2]]
    cc_e = dstpos[eorder2] // SH
    slot_e = dstpos[eorder2] % SH
    tt_e = slot_e // P
    pp_e = slot_e % P
    qe = qq[eorder2]
    le = lq[eorder2]
    cnt4 = kcnt.reshape(NPAD, 4)

    J4 = np.zeros((TPC, 4), np.int64)
    for c in range(NC):
        v = cnt4[c * SH:(c + 1) * SH].reshape(TPC, P, 4)
        J4 = np.maximum(J4, v.max(axis=1))
    groups2 = _group_plan2(J4, cap2)

    Jg_t = np.zeros((TPC, 4), np.int64)
    colbase = np.zeros((TPC, 4), np.int64)
    coff = 0
    for (t0, nt, Jq) in groups2:
        bq = coff
        for q in range(4):
            if Jq[q] > 0:
                for i in range(nt):
                    colbase[t0 + i, q] = bq + i * Jq[q]
                bq += nt * Jq[q]
        Jg_t[t0:t0 + nt] = Jq
        coff = bq
    SLOTS2 = coff

    totJ = Jg_t.sum(1)  # [TPC]
    for c in range(NC):
        m = cc_e == c
        col = colbase[tt_e[m], qe[m]] + rank[m]
        L = np.full((SLOTS2, P), PADIDX, np.int16)
        L[col, pp_e[m]] = le[m].astype(np.int16)
        degc = cnt4[c * SH:(c + 1) * SH].sum(1).reshape(TPC, P)
        npad2 = (totJ[:, None] - degc).astype(np.float32).T.copy()  # [P, TPC]
        # wrapped int16 lists, one contiguous region per (group, quarter) issue
        wparts = []
        cb = 0
        for (t0, nt, Jq) in groups2:
            for q in range(4):
                if Jq[q] == 0:
                    continue
                ncols = nt * Jq[q]
                flat = L[cb:cb + ncols, :].ravel()       # i = col*128 + p
                wparts.append(flat.reshape(-1, 16).T)    # [16, ncols*8]
                cb += ncols
        w16 = np.concatenate(wparts, axis=1)
        assert w16.shape[1] == SLOTS2 * 8, (w16.shape, SLOTS2)
        percore[c]["idx16"] = np.ascontiguousarray(
            np.tile(np.vstack([w16, w16]), (4, 1)).astype(np.int16))
        percore[c]["npad2"] = npad2

    # weight packing
    FIN, H1, C1, CW1 = cfg.FIN, cfg.H1, cfg.C1, cfg.CW1
    waug1 = np.zeros((FIN, CW1 + 2 * H1), np.float32)
    waug1[:, :CW1] = W1
    for h in range(H1):
        waug1[:, CW1 + h] = W1[:, h * C1:(h + 1) * C1] @ a_src1[h]
        waug1[:, CW1 + H1 + h] = W1[:, h * C1:(h + 1) * C1] @ a_dst1[h]
    C2 = cfg.C2
    waug2 = np.zeros((CW1, cfg.WAUG2_C), np.float32)
    waug2[:, :C2] = W2
    waug2[:, C2] = W2[:, :] @ a_src2[0]
    waug2[:, C2 + 1] = W2[:, :] @ a_dst2[0]

    meta = dict(J=[int(j) for j in J], offs=[int(o) for o in offs],
                SLOTS=SLOTS, SLOTS2=SLOTS2, groups2=groups2)
    return percore, waug1, waug2, meta


def dma_gather_nopad(gp, out_ap, in_ap, idxs_ap, num_idxs, elem_size, elem_step,
                     queue_num=0):
    """bass dma_gather (non-transpose, HBM src) minus the elem%256 restriction."""
    assert idxs_ap.dtype == mybir.dt.int16
    assert in_ap.dtype == out_ap.dtype
    assert in_ap.space == bass.MemorySpace.DRAM
    assert ap_utils.ap_is_contiguous(in_ap.ap[1:])
    assert ap_utils.ap_is_contiguous(out_ap.ap[1:])
    assert ap_utils.ap_is_contiguous(idxs_ap.ap[1:])
    assert in_ap.ap[-1][1] == out_ap.ap[-1][1] == elem_size
    assert out_ap.ap[0][1] * out_ap.ap[1][1] == ((num_idxs + 127) // 128) * 128
    assert in_ap.ap[0][0] == elem_step
    stride_bytes = elem_step * mybir.dt.size(in_ap.dtype)
    stride_bytes_256 = stride_bytes // 256
    assert stride_bytes_256 * 256 == stride_bytes and stride_bytes_256 < 256
    _in_ap = gp.lower_ap_dma(in_ap, for_custom_bir_dma=True)
    _idxs_ap = gp.lower_ap(idxs_ap)
    _out_ap = gp.lower_ap(out_ap)
    return gp.add_instruction(
        mybir.InstDMAGatherAnt(
            name=gp.bass.get_next_instruction_name(),
            ins=[*_in_ap, _idxs_ap, gp.lower_val_access(gp.to_reg(num_idxs))],
            outs=[_out_ap],
            transpose=False,
            num_idxs=num_idxs,
            elem_size=elem_size,
            stride_bytes_256=stride_bytes_256,
            gen_mode=0,
            single_packet=False,
            queue_num=queue_num,
            sbuf_tokens_per_rank=0,
            sbuf_free_dim_per_rank=0,
            sbuf_free_dim_pad_per_rank=0,
            sbuf_byte_offset=0,
        )
    )


def build_nc(cfg, meta, group_cap=48, debug=False):
    J, offs = meta["J"], meta["offs"]
    SLOTS, SLOTS2 = meta["SLOTS"], meta["SLOTS2"]
    groups2 = meta["groups2"]
    TPC, NPAD, SH = cfg.TPC, cfg.NPAD, cfg.SH
    QS, QROWS, TABW = cfg.QS, cfg.QROWS, cfg.TABW
    FIN, H1, CW1, C2, F1, F2 = cfg.FIN, cfg.H1, cfg.CW1, cfg.C2, cfg.F1, cfg.F2
    NCOUT = cfg.NCOUT
    groups = _group_plan(J, group_cap)

    nc = bacc.Bacc("TRN2", target_bir_lowering=False, debug=debug,
                   num_devices=cfg.ncores, num_swdge_queues=4)

    # ---- I/O ----
    t_xexpT = nc.dram_tensor("xexpT", [FIN, SLOTS * P], BF16, kind="ExternalInput")
    t_xownT = nc.dram_tensor("xownT", [FIN, SH], BF16, kind="ExternalInput")
    t_waug1 = nc.dram_tensor("waug1", [FIN, CW1 + 2 * H1], BF16, kind="ExternalInput")
    t_waug2 = nc.dram_tensor("waug2", [CW1, cfg.WAUG2_C], BF16, kind="ExternalInput")
    t_wf = nc.dram_tensor("wf", [C2, NCOUT], BF16, kind="ExternalInput")
    t_idx16 = nc.dram_tensor("idx16", [128, SLOTS2 * 8], I16, kind="ExternalInput")
    t_npad1 = nc.dram_tensor("npad1", [P, TPC], F32, kind="ExternalInput")
    t_npad2 = nc.dram_tensor("npad2", [P, TPC], F32, kind="ExternalInput")
    t_b1r = nc.dram_tensor("b1r", [P, CW1], F32, kind="ExternalInput")
    t_b2r = nc.dram_tensor("b2r", [P, C2], F32, kind="ExternalInput")
    t_bfr = nc.dram_tensor("bfr", [P, NCOUT], F32, kind="ExternalInput")
    t_out = nc.dram_tensor("out", [SH, NCOUT], F32, kind="ExternalOutput")

    t_cc_in = nc.dram_tensor("cc_in", [SH, F2], BF16)
    cc_space = "Shared" if cfg.ncores > 4 else "Local"
    t_cc_out = nc.dram_tensor("cc_out", [NPAD, F2], BF16, addr_space=cc_space)
    t_tab2 = nc.dram_tensor("tab2", [4 * QROWS, TABW], BF16)

    with tile.TileContext(nc) as tc:
        with (
            tc.tile_pool(name="res", bufs=1) as res,
            tc.tile_pool(name="pa", bufs=3) as pa,
            tc.tile_pool(name="pb", bufs=2) as pb,
            tc.tile_pool(name="pl2", bufs=2) as pl2,
            tc.tile_pool(name="fin", bufs=2) as fin,
            tc.tile_pool(name="psA", bufs=2, space="PSUM") as psA,
            tc.tile_pool(name="acc", bufs=4, space="PSUM") as accp,
            tc.tile_pool(name="aux", bufs=2, space="PSUM") as auxp,
        ):
            # ---- residents ----
            ident = res.tile([P, P], BF16)
            make_identity(nc, ident[:])
            waug1 = res.tile([FIN, CW1 + 2 * H1], BF16)
            nc.sync.dma_start(waug1[:], t_waug1[:, :])
            waug2 = res.tile([CW1, cfg.WAUG2_C], BF16)
            nc.sync.dma_start(waug2[:], t_waug2[:, :])
            wf = res.tile([C2, NCOUT], BF16)
            nc.sync.dma_start(wf[:], t_wf[:, :])
            npad1 = res.tile([P, TPC], F32)
            nc.sync.dma_start(npad1[:], t_npad1[:, :])
            npad2 = res.tile([P, TPC], F32)
            nc.sync.dma_start(npad2[:], t_npad2[:, :])
            b1r = res.tile([P, CW1], F32)
            nc.sync.dma_start(b1r[:], t_b1r[:, :])
            b2r = res.tile([P, C2], F32)
            nc.sync.dma_start(b2r[:], t_b2r[:, :])
            bfr = res.tile([P, NCOUT], F32)
            nc.sync.dma_start(bfr[:], t_bfr[:, :])
            xownT = res.tile([FIN, SH], BF16)
            nc.sync.dma_start(xownT[:], t_xownT[:, :])
            ad1 = res.tile([P, TPC * H1], F32)
            ad2 = res.tile([P, TPC], F32)
            h2in = res.tile([P, TPC * CW1], BF16)
            h3 = res.tile([P, TPC * C2], BF16)
            zc1 = res.tile([P, TPC * H1], F32)
            zc2 = res.tile([P, TPC], F32)

            # zero rows of tab2 (one per quarter), done early
            zrow = res.tile([1, F2], BF16)
            nc.vector.memset(zrow[:], 0.0)
            for q in range(4):
                r = q * QROWS + QS
                nc.sync.dma_start(t_tab2[r:r + 1, 0:F2], zrow[:])

            # ---- phase A2: alpha_dst1 for own (permuted) nodes ----
            for t in range(TPC):
                ps3 = psA.tile([P, CW1 + 2 * H1], F32, tag="ps_small")
                nc.tensor.matmul(ps3[:, :H1], lhsT=xownT[:, t * P:(t + 1) * P],
                                 rhs=waug1[:, CW1 + H1:CW1 + 2 * H1],
                                 start=True, stop=True)
                nc.vector.tensor_copy(ad1[:, t * H1:(t + 1) * H1], ps3[:, :H1])

            # zc1 = npad1 * bf16(exp(lrelu(ad1)))
            c1b = res.tile([P, TPC * H1], BF16)
            _lrelu_exp(nc, pb, c1b[:], ad1[:], [P, TPC * H1])
            nc.vector.tensor_tensor(
                out=zc1[:].rearrange("p (t h) -> p t h", h=H1),
                in0=c1b[:].rearrange("p (t h) -> p t h", h=H1),
                in1=npad1[:][:, :, None].to_broadcast([P, TPC, H1]),
                op=ALU.mult)

            # ---- layer 1 message passing (host-expanded x + matmul source) ----
            _layer1(nc, tc, pb, fin, accp, psA, groups, offs, t_xexpT, waug1,
                    F1, CW1, H1, ad1, zc1, b1r, ident, h2in)

            # ---- table2 build (own shard) + AllGather + repack ----
            for t in range(TPC):
                tp = auxp.tile([CW1, P], BF16, tag="ps_tp")
                nc.tensor.transpose(tp[:], h2in[:, t * CW1:(t + 1) * CW1], ident[:])
                h2T = pa.tile([CW1, P], BF16, tag="h2T")
                nc.scalar.activation(h2T[:], tp[:], AF.Copy)
                ps2 = psA.tile([P, CW1 + 2 * H1], F32, tag="ps_small")
                nc.tensor.matmul(ps2[:, :cfg.WAUG2_C], lhsT=h2T[:], rhs=waug2[:], start=True, stop=True)
                nc.vector.tensor_copy(ad2[:, t:t + 1], ps2[:, C2 + 1:C2 + 2])
                st2 = pa.tile([P, F2], BF16, tag="st2")
                nc.vector.memset(st2[:], 0.0)
                nc.scalar.activation(st2[:, :C2 + 1], ps2[:, :C2 + 1], AF.Copy)
                nc.sync.dma_start(t_cc_in[t * P:(t + 1) * P, :], st2[:])

            nc.gpsimd.collective_compute(
                "AllGather", ALU.bypass,
                replica_groups=[list(range(cfg.ncores))],
                ins=[t_cc_in.ap().opt()],
                outs=[t_cc_out.ap().opt()],
            )

            # repack cc_out [NPAD, F2] -> tab2 (256B rows, per-quarter layout)
            for q in range(4):
                nc.sync.dma_start(
                    t_tab2[q * QROWS:q * QROWS + QS, 0:F2],
                    t_cc_out[q * QS:(q + 1) * QS, :])

            # zc2 = npad2 * bf16(exp(lrelu(ad2)))   (pad rows are exact zeros)
            c2b = res.tile([P, TPC], BF16)
            _lrelu_exp(nc, pb, c2b[:], ad2[:], [P, TPC])
            nc.vector.tensor_tensor(out=zc2[:], in0=c2b[:], in1=npad2[:], op=ALU.mult)

            # ---- layer 2 message passing (batched dma_gather grid) ----
            _layer2(nc, tc, pl2, fin, accp, groups2, t_idx16, t_tab2, QROWS,
                    TABW, F2, C2, ad2, zc2, b2r, ident, h3)

            # ---- final head: out = h3 @ Wf + bf ----
            GO = 8
            for g in range((TPC + GO - 1) // GO):
                nt = min(GO, TPC - g * GO)
                ost = fin.tile([P, GO * NCOUT], F32, tag="ost")
                for i in range(nt):
                    t = g * GO + i
                    tp = auxp.tile([CW1, P], BF16, tag="ps_tp")
                    nc.tensor.transpose(tp[:C2, :], h3[:, t * C2:(t + 1) * C2], ident[:])
                    h3T = pa.tile([C2, P], BF16, tag="h3T")
                    nc.scalar.activation(h3T[:], tp[:C2, :], AF.Copy)
                    pso = psA.tile([P, CW1 + 2 * H1], F32, tag="ps_small")
                    nc.tensor.matmul(pso[:, :NCOUT], lhsT=h3T[:], rhs=wf[:], start=True, stop=True)
                    nc.vector.tensor_tensor(out=ost[:, i * NCOUT:(i + 1) * NCOUT],
                                            in0=pso[:, :NCOUT], in1=bfr[:], op=ALU.add)
                dst_ap = t_out[g * GO * P:g * GO * P + nt * P, :].rearrange(
                    "(i p) c -> p i c", p=P)
                nc.sync.dma_start(dst_ap, ost[:, :nt * NCOUT].rearrange(
                    "p (i c) -> p i c", c=NCOUT))

    nc.compile()
    return nc


def _lrelu_exp(nc, pool, out_ap, in_ap, shape):
    """out = exp(leakyrelu_0.2(in)) = exp(0.2*(4*relu(in) + in)); out may be bf16."""
    r = pool.tile(shape, F32, tag="lre_r")
    nc.scalar.activation(r[:], in_ap, AF.Relu)
    u = pool.tile(shape, F32, tag="lre_u")
    nc.vector.tensor_scalar(out=u[:], in0=r[:], scalar1=4.0, scalar2=None, op0=ALU.mult)
    nc.vector.tensor_tensor(out=u[:], in0=u[:], in1=in_ap, op=ALU.add)
    nc.scalar.activation(out_ap, u[:], AF.Exp, scale=0.2)


def _finalize_group(nc, fin, out_res, ost, t0, nt, CW, H, br):
    """bias + selu for a group's aggregated outputs -> out_res (bf16)."""
    vb = fin.tile([P, nt * CW], F32, tag="vb")
    nc.vector.tensor_tensor(out=vb[:].rearrange("p (t c) -> p t c", c=CW),
                            in0=ost[:, :nt * CW].rearrange("p (t c) -> p t c", c=CW),
                            in1=br[:][:, None, :].to_broadcast([P, nt, CW]),
                            op=ALU.add)
    rr = fin.tile([P, nt * CW], F32, tag="rr")
    nc.scalar.activation(rr[:], vb[:], AF.Relu)
    w = fin.tile([P, nt * CW], F32, tag="ww")
    nc.vector.tensor_tensor(out=w[:], in0=vb[:], in1=rr[:], op=ALU.subtract)
    e = fin.tile([P, nt * CW], F32, tag="ee")
    nc.scalar.activation(e[:], w[:], AF.Exp)
    nc.vector.tensor_scalar(out=e[:], in0=e[:], scalar1=SELU_ALPHA_SCALE,
                            scalar2=-SELU_ALPHA_SCALE, op0=ALU.mult, op1=ALU.add)
    nc.vector.tensor_scalar(out=rr[:], in0=rr[:], scalar1=SELU_SCALE, scalar2=None,
                            op0=ALU.mult)
    nc.vector.tensor_tensor(out=out_res[:, t0 * CW:(t0 + nt) * CW],
                            in0=e[:], in1=rr[:], op=ALU.add)


def _layer1(nc, tc, pb, fin, accp, psA, groups, offs, t_xexpT, waug, F, CW, H,
            ad, zc, br, ident, out_res):
    """Layer-1 message passing: per-slot matmul source over the uniform-J grid."""
    FV = H + CW
    copy_flip = [0]
    for (t0, nt, Jg) in groups:
        o = offs[t0]
        SJ = nt * Jg
        gath = pb.tile([P, SJ * F], BF16, tag="gath")
        xe = pb.tile([16, SJ * P], BF16, tag="xe")
        nc.sync.dma_start(xe[:], t_xexpT[:, o * P:(o + SJ) * P])
        for s in range(SJ):
            psb = psA.tile([P, 104], F32, tag="ps_small")
            nc.tensor.matmul(psb[:, :F], lhsT=xe[:, s * P:(s + 1) * P],
                             rhs=waug[:, :F], start=True, stop=True)
            if copy_flip[0] % 2 == 0:
                nc.scalar.activation(gath[:, s * F:(s + 1) * F], psb[:, :F], AF.Copy)
            else:
                nc.vector.tensor_copy(gath[:, s * F:(s + 1) * F], psb[:, :F])
            copy_flip[0] += 1
        gv = gath[:].rearrange("p (t j f) -> p t j f", j=Jg, f=F)
        s = pb.tile([P, SJ * H], F32, tag="s")
        s4 = s[:].rearrange("p (t j h) -> p t j h", j=Jg, h=H)
        adv = ad[:].rearrange("p (t h) -> p t h", h=H)[:, t0:t0 + nt, :]
        nc.vector.tensor_tensor(out=s4, in0=gv[:, :, :, CW:CW + H],
                                in1=adv[:, :, None, :].to_broadcast([P, nt, Jg, H]),
                                op=ALU.add)
        r = pb.tile([P, SJ * H], F32, tag="r")
        nc.scalar.activation(r[:], s[:], AF.Relu)
        u = pb.tile([P, SJ * H], F32, tag="u")
        nc.vector.tensor_scalar(out=u[:], in0=r[:], scalar1=4.0, scalar2=None, op0=ALU.mult)
        nc.vector.tensor_tensor(out=u[:], in0=u[:], in1=s[:], op=ALU.add)
        rhs2 = pb.tile([P, SJ * FV], BF16, tag="rhs2")
        r2 = rhs2[:].rearrange("p (t j f) -> p t j f", j=Jg, f=FV)
        nc.scalar.activation(r2[:, :, :, 0:H],
                             u[:].rearrange("p (t j h) -> p t j h", j=Jg, h=H),
                             AF.Exp, scale=0.2)
        nc.vector.tensor_tensor(
            out=r2[:, :, :, H:],
            in0=gv[:, :, :, 0:CW],
            in1=r2[:, :, :, 0:H].to_broadcast([P, nt, Jg, H, CW // H]),
            op=ALU.mult)
        ost = fin.tile([P, nt * CW], F32, tag="ofin")
        for i in range(nt):
            t = t0 + i
            acc = accp.tile([P, FV], F32, tag="agg")
            for j in range(Jg):
                nc.tensor.matmul(acc[:], lhsT=ident[:],
                                 rhs=rhs2[:, (i * Jg + j) * FV:(i * Jg + j + 1) * FV],
                                 start=(j == 0), stop=(j == Jg - 1))
            z = fin.tile([P, H], F32, tag="zf")
            nc.vector.tensor_tensor(out=z[:], in0=acc[:, 0:H],
                                    in1=zc[:, t * H:(t + 1) * H], op=ALU.subtract)
            nc.vector.tensor_scalar(out=z[:], in0=z[:], scalar1=1e-16, scalar2=None,
                                    op0=ALU.add)
            nc.vector.reciprocal(z[:], z[:])
            nc.vector.tensor_tensor(
                out=ost[:, i * CW:(i + 1) * CW].rearrange("p (h c) -> p h c", h=H),
                in0=acc[:, H:].rearrange("p (h c) -> p h c", h=H),
                in1=z[:].to_broadcast([P, H, CW // H]),
                op=ALU.mult)
        _finalize_group(nc, fin, out_res, ost, t0, nt, CW, H, br)


def _layer2(nc, tc, pb, fin, accp, groups2, t_idx16, t_tab2, QROWS,
            TABW, F, CW, ad, zc, br, ident, out_res):
    """Layer-2 message passing: batched dma_gather over the quarter grid."""
    _layer2.qrr = getattr(_layer2, "qrr", [0])
    _layer2.qrr[0] = 0
    H = 1
    FV = H + CW
    woff = 0
    MAXCOL = 64   # <= 8192 idxs per issue
    for (t0, nt, Jq) in groups2:
        nz = [q for q in range(4) if Jq[q] > 0]
        rhs2q = {}
        for qi, q in enumerate(nz):
            ncols = nt * Jq[q]
            gath = pb.tile([P, ncols * F], BF16, tag=f"gath2_{qi}")
            in_ap = t_tab2[q * QROWS:(q + 1) * QROWS, 0:F]
            for a in range(0, ncols, MAXCOL):
                b = min(ncols, a + MAXCOL)
                out_ap = gath[:, a * F:b * F].rearrange("p (c e) -> p c e", e=F)
                ixt = pb.tile([128, (b - a) * 8], I16, tag="ixs")
                nc.sync.dma_start(
                    ixt[:], t_idx16[:, woff + a * 8:woff + b * 8])
                dma_gather_nopad(nc.gpsimd, out_ap, in_ap, ixt[:],
                                 (b - a) * P, F, TABW,
                                 queue_num=_layer2.qrr[0] % 4)
                _layer2.qrr[0] += 1
            # scores for this quarter block (uniform J within block)
            gv = gath[:].rearrange("p (t j f) -> p t j f", j=Jq[q], f=F)
            s = pb.tile([P, ncols * H], F32, tag=f"s2_{qi}")
            s4 = s[:].rearrange("p (t j h) -> p t j h", j=Jq[q], h=H)
            adv = ad[:][:, t0:t0 + nt, None]
            nc.vector.tensor_tensor(out=s4, in0=gv[:, :, :, CW:CW + H],
                                    in1=adv.to_broadcast([P, nt, Jq[q], H]),
                                    op=ALU.add)
            r = pb.tile([P, ncols * H], F32, tag=f"r2_{qi}")
            nc.scalar.activation(r[:], s[:], AF.Relu)
            u = pb.tile([P, ncols * H], F32, tag=f"u2_{qi}")
            nc.vector.tensor_scalar(out=u[:], in0=r[:], scalar1=4.0, scalar2=None,
                                    op0=ALU.mult)
            nc.vector.tensor_tensor(out=u[:], in0=u[:], in1=s[:], op=ALU.add)
            rhs2 = pb.tile([P, ncols * FV], BF16, tag=f"rhs2b_{qi}")
            r2 = rhs2[:].rearrange("p (t j f) -> p t j f", j=Jq[q], f=FV)
            nc.scalar.activation(r2[:, :, :, 0:H],
                                 u[:].rearrange("p (t j h) -> p t j h", j=Jq[q], h=H),
                                 AF.Exp, scale=0.2)
            nc.vector.tensor_tensor(
                out=r2[:, :, :, H:],
                in0=gv[:, :, :, 0:CW],
                in1=r2[:, :, :, 0:H].to_broadcast([P, nt, Jq[q], H, CW // H]),
                op=ALU.mult)
            rhs2q[q] = rhs2
            woff += ncols * 8
        ost = fin.tile([P, nt * CW], F32, tag="ofin2")
        for i in range(nt):
            t = t0 + i
            acc = accp.tile([P, FV], F32, tag="agg")
            for qi, q in enumerate(nz):
                for j in range(Jq[q]):
                    col = i * Jq[q] + j
                    nc.tensor.matmul(
                        acc[:], lhsT=ident[:],
                        rhs=rhs2q[q][:, col * FV:(col + 1) * FV],
                        start=(qi == 0 and j == 0),
                        stop=(qi == len(nz) - 1 and j == Jq[q] - 1))
            z = fin.tile([P, H], F32, tag="zf2")
            nc.vector.tensor_tensor(out=z[:], in0=acc[:, 0:H],
                                    in1=zc[:, t * H:(t + 1) * H], op=ALU.subtract)
            nc.vector.tensor_scalar(out=z[:], in0=z[:], scalar1=1e-16, scalar2=None,
                                    op0=ALU.add)
            nc.vector.reciprocal(z[:], z[:])
            nc.vector.tensor_tensor(
                out=ost[:, i * CW:(i + 1) * CW].rearrange("p (h c) -> p h c", h=H),
                in0=acc[:, H:].rearrange("p (h c) -> p h c", h=H),
                in1=z[:].to_broadcast([P, H, CW // H]),
                op=ALU.mult)
        _finalize_group(nc, fin, out_res, ost, t0, nt, CW, H, br)


def _make_inputs(cfg, percore, waug1, waug2, inputs):
    x = np.asarray(inputs["x"], np.float32)
    xpad = np.zeros((cfg.NPAD, cfg.FIN), np.float32)
    xpad[:cfg.N] = x
    wf = np.asarray(inputs["Wf"], np.float32).astype(NPBF16)
    b1r = np.broadcast_to(np.asarray(inputs["b1"], np.float32), (P, cfg.CW1)).copy()
    b2r = np.broadcast_to(np.asarray(inputs["b2"], np.float32), (P, cfg.C2)).copy()
    bfr = np.broadcast_to(np.asarray(inputs["bf"], np.float32), (P, cfg.NCOUT)).copy()
    in_maps = []
    for c in range(cfg.ncores):
        pc = percore[c]
        xownT = np.ascontiguousarray(xpad[pc["own"]].T).astype(NPBF16)
        in_maps.append({
            "xexpT": pc["xexpT"], "xownT": xownT,
            "waug1": waug1.astype(NPBF16), "waug2": waug2.astype(NPBF16),
            "wf": wf, "idx16": pc["idx16"],
            "npad1": pc["npad1"], "npad2": pc["npad2"],
            "b1r": b1r, "b2r": b2r, "bfr": bfr,
        })
    return in_maps


def _assemble(cfg, percore, results):
    out = np.zeros((cfg.NPAD, cfg.NCOUT), np.float32)
    for c in range(cfg.ncores):
        out[percore[c]["own"]] = results[c]["out"]
    return out[:cfg.N]


def kernel(**inputs) -> np.ndarray:
    cfg = Cfg(N=100000, E=800000, ncores=8)
    percore, waug1, waug2, meta = preprocess(
        cfg,
        np.asarray(inputs["x"], np.float32),
        np.asarray(inputs["edge_index"]),
        np.asarray(inputs["W1"], np.float32),
        np.asarray(inputs["a_src1"], np.float32),
        np.asarray(inputs["a_dst1"], np.float32),
        np.asarray(inputs["W2"], np.float32),
        np.asarray(inputs["a_src2"], np.float32),
        np.asarray(inputs["a_dst2"], np.float32),
    )
    nc = build_nc(cfg, meta)
    in_maps = _make_inputs(cfg, percore, waug1, waug2, inputs)
    res = run_bass_kernel_spmd(nc, in_maps, core_ids=list(range(cfg.ncores)))
    return _assemble(cfg, percore, res.results)


if __name__ == "__main__":
    import reference as R
    inputs = R.setup_inputs()
    out = kernel(**{k: np.asarray(v) for k, v in inputs.items()})
    print("out", out.shape, out.dtype)


# revision 23
# speedup vs baseline: 1.5210x; 1.5210x over previous
"""Trainium2 Bass kernel for nn_GAT_1675037246077 (2-layer GAT + linear head).

Strategy (8 NeuronCores, SPMD single NEFF):
 - Destination-sharded: core c owns SH=12544 dst nodes; nodes assigned to cores
   by a host-side greedy that spreads each node's in-edge sources evenly across
   the 4 core-pairs ("quarters"), then within each core sorted by in-degree and
   packed into 98 tiles of 128 nodes.
 - Layer 1: host expands x into edge-slot order (xexpT); per-slot h1 = xe @ Waug1
   on the PE; scores exp(leakyrelu) with alpha_dst per-partition; weighted
   aggregation via identity-matmul PSUM accumulation; exact pad-slot z corrections.
 - Layer 2: table2[row=pos(node)] = [h2|as2] built shard-wise, AllGather'd,
   repacked into a 256B-row table with one explicit zero row per quarter.
   Edge gathering uses dma_gather (InstDMAGatherAnt): one issue per
   (tile-group, quarter) with int16 quarter-local row indices -> the gather's
   native output layout IS the slot grid. Per-quarter slot counts are uniform
   within a group (J maxed over the group's tiles); the greedy quarter balance
   keeps the padding small.
 - No segment-max: scores are bounded (|e| < 2), exp is safe.
"""
import numpy as np
import ml_dtypes

from concourse import mybir, tile, bacc
import concourse.bass as bass
from concourse import ap_utils
from concourse.bass_utils import run_bass_kernel_spmd
from concourse.masks import make_identity

P = 128
AF = mybir.ActivationFunctionType
ALU = mybir.AluOpType
BF16 = mybir.dt.bfloat16
F32 = mybir.dt.float32
I16 = mybir.dt.int16
NPBF16 = ml_dtypes.bfloat16

SELU_SCALE = 1.0507009873554805
SELU_ALPHA_SCALE = 1.7580993408473766  # scale * alpha


class Cfg:
    def __init__(self, N, E, ncores, fin=16, h1=3, c1=32, c2=32, ncout=16):
        self.N, self.E, self.ncores = N, E, ncores
        self.FIN, self.H1, self.C1, self.C2, self.NCOUT = fin, h1, c1, c2, ncout
        self.CW1 = h1 * c1              # 96
        self.F1 = self.CW1 + h1         # table1 cols (99)
        self.F2 = c2 + 1 + 3            # table2 cols (36): [h2|as2|pad3]
        self.SH = ((N + ncores - 1) // ncores + P - 1) // P * P
        self.TPC = self.SH // P
        self.NPAD = self.SH * ncores
        self.WAUG2_C = c2 + 2           # [W2(32)|as2|ad2]
        self.QS = 2 * self.SH           # quarter size (rows), int16-addressable
        self.QROWS = self.QS + 1        # + zero row
        self.TABW = 128                 # padded table row width (bf16) = 256B


def _group_plan(J, cap):
    groups = []
    t = 0
    n = len(J)
    while t < n:
        j = J[t]
        nt = 1
        while t + nt < n and J[t + nt] == j and (nt + 1) * j <= cap and nt < 6:
            nt += 1
        groups.append((t, nt, int(j)))
        t += nt
    return groups


def _group_plan2(J2, cap, max_nt=16):
    """Group consecutive tiles; per-quarter J maxed over the group.
    Returns [(t0, nt, (Jh0, Jh1))]."""
    groups = []
    t = 0
    n = len(J2)
    tilesum = J2.sum(axis=1)
    while t < n:
        nt = 1
        while t + nt < n and nt < max_nt:
            m = np.maximum.reduce(J2[t:t + nt + 1], axis=0)
            padded = (nt + 1) * int(m.sum())
            if padded > cap or padded > 1.15 * int(tilesum[t:t + nt + 1].sum()) + 6:
                break
            nt += 1
        m = np.maximum.reduce(J2[t:t + nt], axis=0)
        groups.append((t, nt, tuple(int(v) for v in m)))
        t += nt
    return groups


def _assign_quarters(src, dst, N, NPAD, QS, npasses=3):
    """Greedy (with refinement passes): assign each node to one of 4 quarters
    (capacity QS) so every dst node's in-edge srcs spread evenly; penalize
    exceeding the per-dst per-quarter target ceil(deg/4)."""
    deg = np.bincount(dst, minlength=NPAD)
    tgt = np.ceil(deg / 4).astype(np.float32)
    sdeg = np.bincount(src, minlength=NPAD).astype(np.int64)
    sstart = np.zeros(NPAD + 1, np.int64)
    np.cumsum(sdeg, out=sstart[1:])
    dst_by_src = dst[np.argsort(src, kind="stable")]
    cnt = np.zeros((NPAD, 4), np.float32)
    cap = np.full(4, QS, np.int64)
    qa = np.full(NPAD, -1, np.int8)
    rng = np.random.default_rng(0)
    nodes = rng.permutation(N)
    B = 1024
    for pas in range(npasses):
        for b0 in range(0, N, B):
            batch = nodes[b0:b0 + B]
            lens = sdeg[batch]
            tot = int(lens.sum())
            sc = np.zeros((len(batch), 4), np.float32)
            dd = seg = None
            if tot:
                ends = np.cumsum(lens)
                base = np.repeat(sstart[batch] - (ends - lens), lens)
                dd = dst_by_src[base + np.arange(tot)]
                seg = np.repeat(np.arange(len(batch)), lens)
            if pas > 0:
                old = qa[batch].astype(np.int64)
                cap += np.bincount(old, minlength=4)
                if dd is not None:
                    np.add.at(cnt, (dd, old[seg]), -1.0)
            if dd is not None:
                c = cnt[dd]
                np.add.at(sc, seg,
                          100.0 * np.maximum(0, c + 1 - tgt[dd][:, None]) + c)
            pen = np.where(cap > 0, (QS - cap).astype(np.float32) * 1e-4,
                           np.float32(np.inf))
            choice = np.argmin(sc + pen[None, :], axis=1).astype(np.int64)
            counts = np.bincount(choice, minlength=4)
            while np.any(counts > cap):
                q = int(np.argmax(counts - cap))
                movable = np.where(choice == q)[0]
                room = cap - counts
                q2 = int(np.argmax(room))
                nmove = min(int(counts[q] - cap[q]), int(room[q2]))
                choice[movable[-nmove:]] = q2
                counts = np.bincount(choice, minlength=4)
            qa[batch] = choice
            cap -= counts
            if dd is not None:
                np.add.at(cnt, (dd, choice[seg]), 1.0)
    pads = np.arange(N, NPAD)
    fill = np.repeat(np.arange(4), np.maximum(cap, 0))
    qa[pads] = fill[:len(pads)].astype(np.int8)
    return qa, cnt.astype(np.int32)


def preprocess(cfg, x, edge_index, W1, a_src1, a_dst1, W2, a_src2, a_dst2,
               cap2=160):
    N, E, NC = cfg.N, cfg.E, cfg.ncores
    SH, TPC, NPAD, QS = cfg.SH, cfg.TPC, cfg.NPAD, cfg.QS

    loops = np.arange(N, dtype=np.int64)
    src = np.concatenate([edge_index[0].astype(np.int64), loops])
    dst = np.concatenate([edge_index[1].astype(np.int64), loops])

    deg = np.bincount(dst, minlength=NPAD)

    qa, qcnt = _assign_quarters(src, dst, N, NPAD, QS)

    # per-quarter (deg, quarter-profile) sort -> aligned tiles across cores;
    # alternate ranks between the quarter's two cores
    perms = [None] * NC
    pos = np.empty(NPAD, np.int64)
    for q in range(4):
        nq = np.where(qa == q)[0]
        cq = qcnt[nq]
        order = np.lexsort((cq[:, 3], cq[:, 2], cq[:, 1], cq[:, 0], -deg[nq]))
        nq = nq[order]
        for par in range(2):
            c = 2 * q + par
            perm = nq[par::2]
            assert len(perm) == SH
            perms[c] = perm
            pos[perm] = c * SH + np.arange(SH)

    Jt_all = np.zeros((NC, TPC), np.int64)
    for c in range(NC):
        Jt_all[c] = deg[perms[c]].reshape(TPC, P).max(1)
    J = np.maximum(Jt_all.max(0), 1)
    SLOTS = int(J.sum())

    # CSR of edges by dst (node ids)
    e_order = np.argsort(dst, kind="stable")
    src_sorted = src[e_order]
    starts = np.zeros(NPAD + 1, np.int64)
    np.cumsum(deg, out=starts[1:])

    offs = np.zeros(TPC + 1, np.int64)
    np.cumsum(J, out=offs[1:])

    percore = []
    for c in range(NC):
        perm = perms[c]
        idx1 = np.full((P, SLOTS), NPAD - 1, np.int32)  # pad -> x=0 node
        npad1 = np.zeros((P, TPC), np.float32)
        for t in range(TPC):
            jt = int(J[t])
            o = int(offs[t])
            for p in range(P):
                node = perm[t * P + p]
                dg = int(deg[node])
                s0 = int(starts[node])
                idx1[p, o:o + dg] = src_sorted[s0:s0 + dg]
                npad1[p, t] = jt - dg
        percore.append(dict(idx1=idx1, npad1=npad1, own=perm.astype(np.int64)))

    # host-side L1 expansion: x rows in edge-slot order, transposed
    xpad = np.zeros((NPAD, cfg.FIN), np.float32)
    xpad[:N] = x
    xpadT_bf = np.ascontiguousarray(xpad.T).astype(NPBF16)
    for c in range(NC):
        cols = percore[c]["idx1"].T.ravel()
        percore[c]["xexpT"] = np.ascontiguousarray(xpadT_bf[:, cols])

    # ---- layer-2 quarter grid (int16 local rows, pad -> zero row QS) ----
    PADIDX = np.int16(QS)
    srcpos = pos[src]
    qq = srcpos // QS
    lq = srcpos % QS
    dstpos = pos[dst]
    okey = dstpos * 4 + qq
    eorder2 = np.argsort(okey, kind="stable")
    kcnt = np.bincount(okey, minlength=NPAD * 4)
    kstart = np.zeros(NPAD * 4 + 1, np.int64)
    np.cumsum(kcnt, out=kstart[1:])
    rank = np.arange(len(eorder2)) - kstart[okey[eorder2]]
    cc_e = dstpos[eorder2] // SH
    slot_e = dstpos[eorder2] % SH
    tt_e = slot_e // P
    pp_e = slot_e % P
    qe = qq[eorder2]
    le = lq[eorder2]
    cnt4 = kcnt.reshape(NPAD, 4)

    J4 = np.zeros((TPC, 4), np.int64)
    for c in range(NC):
        v = cnt4[c * SH:(c + 1) * SH].reshape(TPC, P, 4)
        J4 = np.maximum(J4, v.max(axis=1))
    groups2 = _group_plan2(J4, cap2)

    Jg_t = np.zeros((TPC, 4), np.int64)
    colbase = np.zeros((TPC, 4), np.int64)
    coff = 0
    for (t0, nt, Jq) in groups2:
        bq = coff
        for q in range(4):
            if Jq[q] > 0:
                for i in range(nt):
                    colbase[t0 + i, q] = bq + i * Jq[q]
                bq += nt * Jq[q]
        Jg_t[t0:t0 + nt] = Jq
        coff = bq
    SLOTS2 = coff

    totJ = Jg_t.sum(1)  # [TPC]
    for c in range(NC):
        m = cc_e == c
        col = colbase[tt_e[m], qe[m]] + rank[m]
        L = np.full((SLOTS2, P), PADIDX, np.int16)
        L[col, pp_e[m]] = le[m].astype(np.int16)
        degc = cnt4[c * SH:(c + 1) * SH].sum(1).reshape(TPC, P)
        npad2 = (totJ[:, None] - degc).astype(np.float32).T.copy()  # [P, TPC]
        # wrapped int16 lists, one contiguous region per (group, quarter) issue
        wparts = []
        cb = 0
        for (t0, nt, Jq) in groups2:
            for q in range(4):
                if Jq[q] == 0:
                    continue
                ncols = nt * Jq[q]
                flat = L[cb:cb + ncols, :].ravel()       # i = col*128 + p
                wparts.append(flat.reshape(-1, 16).T)    # [16, ncols*8]
                cb += ncols
        w16 = np.concatenate(wparts, axis=1)
        assert w16.shape[1] == SLOTS2 * 8, (w16.shape, SLOTS2)
        percore[c]["idx16"] = np.ascontiguousarray(
            np.tile(np.vstack([w16, w16]), (4, 1)).astype(np.int16))
        percore[c]["npad2"] = npad2

    # weight packing
    FIN, H1, C1, CW1 = cfg.FIN, cfg.H1, cfg.C1, cfg.CW1
    waug1 = np.zeros((FIN, CW1 + 2 * H1), np.float32)
    waug1[:, :CW1] = W1
    for h in range(H1):
        waug1[:, CW1 + h] = W1[:, h * C1:(h + 1) * C1] @ a_src1[h]
        waug1[:, CW1 + H1 + h] = W1[:, h * C1:(h + 1) * C1] @ a_dst1[h]
    C2 = cfg.C2
    waug2 = np.zeros((CW1, cfg.WAUG2_C), np.float32)
    waug2[:, :C2] = W2
    waug2[:, C2] = W2[:, :] @ a_src2[0]
    waug2[:, C2 + 1] = W2[:, :] @ a_dst2[0]

    meta = dict(J=[int(j) for j in J], offs=[int(o) for o in offs],
                SLOTS=SLOTS, SLOTS2=SLOTS2, groups2=groups2)
    return percore, waug1, waug2, meta


def dma_gather_nopad(gp, out_ap, in_ap, idxs_ap, num_idxs, elem_size, elem_step,
                     queue_num=0):
    """bass dma_gather (non-transpose, HBM src) minus the elem%256 restriction."""
    assert idxs_ap.dtype == mybir.dt.int16
    assert in_ap.dtype == out_ap.dtype
    assert in_ap.space == bass.MemorySpace.DRAM
    assert ap_utils.ap_is_contiguous(in_ap.ap[1:])
    assert ap_utils.ap_is_contiguous(out_ap.ap[1:])
    assert ap_utils.ap_is_contiguous(idxs_ap.ap[1:])
    assert in_ap.ap[-1][1] == out_ap.ap[-1][1] == elem_size
    assert out_ap.ap[0][1] * out_ap.ap[1][1] == ((num_idxs + 127) // 128) * 128
    assert in_ap.ap[0][0] == elem_step
    stride_bytes = elem_step * mybir.dt.size(in_ap.dtype)
    stride_bytes_256 = stride_bytes // 256
    assert stride_bytes_256 * 256 == stride_bytes and stride_bytes_256 < 256
    _in_ap = gp.lower_ap_dma(in_ap, for_custom_bir_dma=True)
    _idxs_ap = gp.lower_ap(idxs_ap)
    _out_ap = gp.lower_ap(out_ap)
    return gp.add_instruction(
        mybir.InstDMAGatherAnt(
            name=gp.bass.get_next_instruction_name(),
            ins=[*_in_ap, _idxs_ap, gp.lower_val_access(gp.to_reg(num_idxs))],
            outs=[_out_ap],
            transpose=False,
            num_idxs=num_idxs,
            elem_size=elem_size,
            stride_bytes_256=stride_bytes_256,
            gen_mode=0,
            single_packet=False,
            queue_num=queue_num,
            sbuf_tokens_per_rank=0,
            sbuf_free_dim_per_rank=0,
            sbuf_free_dim_pad_per_rank=0,
            sbuf_byte_offset=0,
        )
    )


def build_nc(cfg, meta, group_cap=48, debug=False):
    J, offs = meta["J"], meta["offs"]
    SLOTS, SLOTS2 = meta["SLOTS"], meta["SLOTS2"]
    groups2 = meta["groups2"]
    TPC, NPAD, SH = cfg.TPC, cfg.NPAD, cfg.SH
    QS, QROWS, TABW = cfg.QS, cfg.QROWS, cfg.TABW
    FIN, H1, CW1, C2, F1, F2 = cfg.FIN, cfg.H1, cfg.CW1, cfg.C2, cfg.F1, cfg.F2
    NCOUT = cfg.NCOUT
    groups = _group_plan(J, group_cap)

    nc = bacc.Bacc("TRN2", target_bir_lowering=False, debug=debug,
                   num_devices=cfg.ncores, num_swdge_queues=4)

    # ---- I/O ----
    t_xexpT = nc.dram_tensor("xexpT", [FIN, SLOTS * P], BF16, kind="ExternalInput")
    t_xownT = nc.dram_tensor("xownT", [FIN, SH], BF16, kind="ExternalInput")
    t_waug1 = nc.dram_tensor("waug1", [FIN, CW1 + 2 * H1], BF16, kind="ExternalInput")
    t_waug2 = nc.dram_tensor("waug2", [CW1, cfg.WAUG2_C], BF16, kind="ExternalInput")
    t_wf = nc.dram_tensor("wf", [C2, NCOUT], BF16, kind="ExternalInput")
    t_idx16 = nc.dram_tensor("idx16", [128, SLOTS2 * 8], I16, kind="ExternalInput")
    t_npad1 = nc.dram_tensor("npad1", [P, TPC], F32, kind="ExternalInput")
    t_npad2 = nc.dram_tensor("npad2", [P, TPC], F32, kind="ExternalInput")
    t_b1r = nc.dram_tensor("b1r", [P, CW1], F32, kind="ExternalInput")
    t_b2r = nc.dram_tensor("b2r", [P, C2], F32, kind="ExternalInput")
    t_bfr = nc.dram_tensor("bfr", [P, NCOUT], F32, kind="ExternalInput")
    t_out = nc.dram_tensor("out", [SH, NCOUT], F32, kind="ExternalOutput")

    t_cc_in = nc.dram_tensor("cc_in", [SH, F2], BF16)
    cc_space = "Shared" if cfg.ncores > 4 else "Local"
    t_cc_out = nc.dram_tensor("cc_out", [NPAD, F2], BF16, addr_space=cc_space)
    t_tab2 = nc.dram_tensor("tab2", [4 * QROWS, TABW], BF16)

    with tile.TileContext(nc) as tc:
        with (
            tc.tile_pool(name="res", bufs=1) as res,
            tc.tile_pool(name="pa", bufs=3) as pa,
            tc.tile_pool(name="pb", bufs=2) as pb,
            tc.tile_pool(name="pl2", bufs=2) as pl2,
            tc.tile_pool(name="fin", bufs=2) as fin,
            tc.tile_pool(name="psA", bufs=2, space="PSUM") as psA,
            tc.tile_pool(name="acc", bufs=4, space="PSUM") as accp,
            tc.tile_pool(name="aux", bufs=2, space="PSUM") as auxp,
        ):
            # ---- residents ----
            ident = res.tile([P, P], BF16)
            make_identity(nc, ident[:])
            waug1 = res.tile([FIN, CW1 + 2 * H1], BF16)
            nc.sync.dma_start(waug1[:], t_waug1[:, :])
            waug2 = res.tile([CW1, cfg.WAUG2_C], BF16)
            nc.sync.dma_start(waug2[:], t_waug2[:, :])
            wf = res.tile([C2, NCOUT], BF16)
            nc.sync.dma_start(wf[:], t_wf[:, :])
            npad1 = res.tile([P, TPC], F32)
            nc.sync.dma_start(npad1[:], t_npad1[:, :])
            npad2 = res.tile([P, TPC], F32)
            nc.sync.dma_start(npad2[:], t_npad2[:, :])
            b1r = res.tile([P, CW1], F32)
            nc.sync.dma_start(b1r[:], t_b1r[:, :])
            b2r = res.tile([P, C2], F32)
            nc.sync.dma_start(b2r[:], t_b2r[:, :])
            bfr = res.tile([P, NCOUT], F32)
            nc.sync.dma_start(bfr[:], t_bfr[:, :])
            xownT = res.tile([FIN, SH], BF16)
            nc.sync.dma_start(xownT[:], t_xownT[:, :])
            ad1 = res.tile([P, TPC * H1], F32)
            ad2 = res.tile([P, TPC], F32)
            h2in = res.tile([P, TPC * CW1], BF16)
            h3 = res.tile([P, TPC * C2], BF16)
            zc1 = res.tile([P, TPC * H1], F32)
            zc2 = res.tile([P, TPC], F32)

            # zero rows of tab2 (one per quarter), done early
            zrow = res.tile([1, F2], BF16)
            nc.vector.memset(zrow[:], 0.0)
            for q in range(4):
                r = q * QROWS + QS
                nc.sync.dma_start(t_tab2[r:r + 1, 0:F2], zrow[:])

            # ---- phase A2: alpha_dst1 for own (permuted) nodes ----
            for t in range(TPC):
                ps3 = psA.tile([P, CW1 + 2 * H1], F32, tag="ps_small")
                nc.tensor.matmul(ps3[:, :H1], lhsT=xownT[:, t * P:(t + 1) * P],
                                 rhs=waug1[:, CW1 + H1:CW1 + 2 * H1],
                                 start=True, stop=True)
                nc.vector.tensor_copy(ad1[:, t * H1:(t + 1) * H1], ps3[:, :H1])

            # zc1 = npad1 * bf16(exp(lrelu(ad1)))
            c1b = res.tile([P, TPC * H1], BF16)
            _lrelu_exp(nc, pb, c1b[:], ad1[:], [P, TPC * H1])
            nc.vector.tensor_tensor(
                out=zc1[:].rearrange("p (t h) -> p t h", h=H1),
                in0=c1b[:].rearrange("p (t h) -> p t h", h=H1),
                in1=npad1[:][:, :, None].to_broadcast([P, TPC, H1]),
                op=ALU.mult)

            # ---- layer 1 message passing (host-expanded x + matmul source) ----
            _layer1(nc, tc, pb, fin, accp, psA, groups, offs, t_xexpT, waug1,
                    F1, CW1, H1, ad1, zc1, b1r, ident, h2in)

            # ---- table2 build (own shard) + AllGather + repack ----
            for t in range(TPC):
                tp = auxp.tile([CW1, P], BF16, tag="ps_tp")
                nc.tensor.transpose(tp[:], h2in[:, t * CW1:(t + 1) * CW1], ident[:])
                h2T = pa.tile([CW1, P], BF16, tag="h2T")
                nc.scalar.activation(h2T[:], tp[:], AF.Copy)
                ps2 = psA.tile([P, CW1 + 2 * H1], F32, tag="ps_small")
                nc.tensor.matmul(ps2[:, :cfg.WAUG2_C], lhsT=h2T[:], rhs=waug2[:], start=True, stop=True)
                nc.vector.tensor_copy(ad2[:, t:t + 1], ps2[:, C2 + 1:C2 + 2])
                st2 = pa.tile([P, F2], BF16, tag="st2")
                nc.vector.memset(st2[:], 0.0)
                nc.scalar.activation(st2[:, :C2 + 1], ps2[:, :C2 + 1], AF.Copy)
                nc.sync.dma_start(t_cc_in[t * P:(t + 1) * P, :], st2[:])

            nc.gpsimd.collective_compute(
                "AllGather", ALU.bypass,
                replica_groups=[list(range(cfg.ncores))],
                ins=[t_cc_in.ap().opt()],
                outs=[t_cc_out.ap().opt()],
            )

            # repack cc_out [NPAD, F2] -> tab2 (256B rows, per-quarter layout)
            for q in range(4):
                nc.sync.dma_start(
                    t_tab2[q * QROWS:q * QROWS + QS, 0:F2],
                    t_cc_out[q * QS:(q + 1) * QS, :])

            # zc2 = npad2 * bf16(exp(lrelu(ad2)))   (pad rows are exact zeros)
            c2b = res.tile([P, TPC], BF16)
            _lrelu_exp(nc, pb, c2b[:], ad2[:], [P, TPC])
            nc.vector.tensor_tensor(out=zc2[:], in0=c2b[:], in1=npad2[:], op=ALU.mult)

            # ---- layer 2 message passing (batched dma_gather grid) ----
            _layer2(nc, tc, pl2, fin, accp, groups2, t_idx16, t_tab2, QROWS,
                    TABW, F2, C2, ad2, zc2, b2r, ident, h3)

            # ---- final head: out = h3 @ Wf + bf ----
            GO = 8
            for g in range((TPC + GO - 1) // GO):
                nt = min(GO, TPC - g * GO)
                ost = fin.tile([P, GO * NCOUT], F32, tag="ost")
                for i in range(nt):
                    t = g * GO + i
                    tp = auxp.tile([CW1, P], BF16, tag="ps_tp")
                    nc.tensor.transpose(tp[:C2, :], h3[:, t * C2:(t + 1) * C2], ident[:])
                    h3T = pa.tile([C2, P], BF16, tag="h3T")
                    nc.scalar.activation(h3T[:], tp[:C2, :], AF.Copy)
                    pso = psA.tile([P, CW1 + 2 * H1], F32, tag="ps_small")
                    nc.tensor.matmul(pso[:, :NCOUT], lhsT=h3T[:], rhs=wf[:], start=True, stop=True)
                    nc.vector.tensor_tensor(out=ost[:, i * NCOUT:(i + 1) * NCOUT],
                                            in0=pso[:, :NCOUT], in1=bfr[:], op=ALU.add)
                dst_ap = t_out[g * GO * P:g * GO * P + nt * P, :].rearrange(
                    "(i p) c -> p i c", p=P)
                nc.sync.dma_start(dst_ap, ost[:, :nt * NCOUT].rearrange(
                    "p (i c) -> p i c", c=NCOUT))

    nc.compile()
    return nc


def _lrelu_exp(nc, pool, out_ap, in_ap, shape):
    """out = exp(leakyrelu_0.2(in)) = exp(0.2*(4*relu(in) + in)); out may be bf16."""
    r = pool.tile(shape, F32, tag="lre_r")
    nc.scalar.activation(r[:], in_ap, AF.Relu)
    u = pool.tile(shape, F32, tag="lre_u")
    nc.vector.tensor_scalar(out=u[:], in0=r[:], scalar1=4.0, scalar2=None, op0=ALU.mult)
    nc.vector.tensor_tensor(out=u[:], in0=u[:], in1=in_ap, op=ALU.add)
    nc.scalar.activation(out_ap, u[:], AF.Exp, scale=0.2)


def _finalize_group(nc, fin, out_res, ost, t0, nt, CW, H, br):
    """bias + selu for a group's aggregated outputs -> out_res (bf16)."""
    vb = fin.tile([P, nt * CW], F32, tag="vb")
    nc.vector.tensor_tensor(out=vb[:].rearrange("p (t c) -> p t c", c=CW),
                            in0=ost[:, :nt * CW].rearrange("p (t c) -> p t c", c=CW),
                            in1=br[:][:, None, :].to_broadcast([P, nt, CW]),
                            op=ALU.add)
    rr = fin.tile([P, nt * CW], F32, tag="rr")
    nc.scalar.activation(rr[:], vb[:], AF.Relu)
    w = fin.tile([P, nt * CW], F32, tag="ww")
    nc.vector.tensor_tensor(out=w[:], in0=vb[:], in1=rr[:], op=ALU.subtract)
    e = fin.tile([P, nt * CW], F32, tag="ee")
    nc.scalar.activation(e[:], w[:], AF.Exp)
    nc.vector.tensor_scalar(out=e[:], in0=e[:], scalar1=SELU_ALPHA_SCALE,
                            scalar2=-SELU_ALPHA_SCALE, op0=ALU.mult, op1=ALU.add)
    nc.vector.tensor_scalar(out=rr[:], in0=rr[:], scalar1=SELU_SCALE, scalar2=None,
                            op0=ALU.mult)
    nc.vector.tensor_tensor(out=out_res[:, t0 * CW:(t0 + nt) * CW],
                            in0=e[:], in1=rr[:], op=ALU.add)


def _layer1(nc, tc, pb, fin, accp, psA, groups, offs, t_xexpT, waug, F, CW, H,
            ad, zc, br, ident, out_res):
    """Layer-1 message passing: per-slot matmul source over the uniform-J grid."""
    FV = H + CW
    copy_flip = [0]
    for (t0, nt, Jg) in groups:
        o = offs[t0]
        SJ = nt * Jg
        gath = pb.tile([P, SJ * F], BF16, tag="gath")
        xe = pb.tile([16, SJ * P], BF16, tag="xe")
        nc.sync.dma_start(xe[:], t_xexpT[:, o * P:(o + SJ) * P])
        for s in range(SJ):
            psb = psA.tile([P, 104], F32, tag="ps_small")
            nc.tensor.matmul(psb[:, :F], lhsT=xe[:, s * P:(s + 1) * P],
                             rhs=waug[:, :F], start=True, stop=True)
            if copy_flip[0] % 2 == 0:
                nc.scalar.activation(gath[:, s * F:(s + 1) * F], psb[:, :F], AF.Copy)
            else:
                nc.vector.tensor_copy(gath[:, s * F:(s + 1) * F], psb[:, :F])
            copy_flip[0] += 1
        gv = gath[:].rearrange("p (t j f) -> p t j f", j=Jg, f=F)
        s = pb.tile([P, SJ * H], F32, tag="s")
        s4 = s[:].rearrange("p (t j h) -> p t j h", j=Jg, h=H)
        adv = ad[:].rearrange("p (t h) -> p t h", h=H)[:, t0:t0 + nt, :]
        nc.vector.tensor_tensor(out=s4, in0=gv[:, :, :, CW:CW + H],
                                in1=adv[:, :, None, :].to_broadcast([P, nt, Jg, H]),
                                op=ALU.add)
        r = pb.tile([P, SJ * H], F32, tag="r")
        nc.scalar.activation(r[:], s[:], AF.Relu)
        u = pb.tile([P, SJ * H], F32, tag="u")
        nc.vector.tensor_scalar(out=u[:], in0=r[:], scalar1=4.0, scalar2=None, op0=ALU.mult)
        nc.vector.tensor_tensor(out=u[:], in0=u[:], in1=s[:], op=ALU.add)
        rhs2 = pb.tile([P, SJ * FV], BF16, tag="rhs2")
        r2 = rhs2[:].rearrange("p (t j f) -> p t j f", j=Jg, f=FV)
        nc.scalar.activation(r2[:, :, :, 0:H],
                             u[:].rearrange("p (t j h) -> p t j h", j=Jg, h=H),
                             AF.Exp, scale=0.2)
        nc.vector.tensor_tensor(
            out=r2[:, :, :, H:],
            in0=gv[:, :, :, 0:CW],
            in1=r2[:, :, :, 0:H].to_broadcast([P, nt, Jg, H, CW // H]),
            op=ALU.mult)
        ost = fin.tile([P, nt * CW], F32, tag="ofin")
        for i in range(nt):
            t = t0 + i
            acc = accp.tile([P, FV], F32, tag="agg")
            for j in range(Jg):
                nc.tensor.matmul(acc[:], lhsT=ident[:],
                                 rhs=rhs2[:, (i * Jg + j) * FV:(i * Jg + j + 1) * FV],
                                 start=(j == 0), stop=(j == Jg - 1))
            z = fin.tile([P, H], F32, tag="zf")
            nc.vector.tensor_tensor(out=z[:], in0=acc[:, 0:H],
                                    in1=zc[:, t * H:(t + 1) * H], op=ALU.subtract)
            nc.vector.tensor_scalar(out=z[:], in0=z[:], scalar1=1e-16, scalar2=None,
                                    op0=ALU.add)
            nc.vector.reciprocal(z[:], z[:])
            nc.vector.tensor_tensor(
                out=ost[:, i * CW:(i + 1) * CW].rearrange("p (h c) -> p h c", h=H),
                in0=acc[:, H:].rearrange("p (h c) -> p h c", h=H),
                in1=z[:].to_broadcast([P, H, CW // H]),
                op=ALU.mult)
        _finalize_group(nc, fin, out_res, ost, t0, nt, CW, H, br)


def _layer2(nc, tc, pb, fin, accp, groups2, t_idx16, t_tab2, QROWS,
            TABW, F, CW, ad, zc, br, ident, out_res):
    """Layer-2 message passing: batched dma_gather over the quarter grid."""
    _layer2.qrr = getattr(_layer2, "qrr", [0])
    _layer2.qrr[0] = 0
    H = 1
    FV = H + CW
    woff = 0
    MAXCOL = 64   # <= 8192 idxs per issue
    for (t0, nt, Jq) in groups2:
        nz = [q for q in range(4) if Jq[q] > 0]
        rhs2q = {}
        for qi, q in enumerate(nz):
            ncols = nt * Jq[q]
            gath = pb.tile([P, ncols * F], BF16, tag=f"gath2_{qi}")
            in_ap = t_tab2[q * QROWS:(q + 1) * QROWS, 0:F]
            for a in range(0, ncols, MAXCOL):
                b = min(ncols, a + MAXCOL)
                out_ap = gath[:, a * F:b * F].rearrange("p (c e) -> p c e", e=F)
                ixt = pb.tile([128, (b - a) * 8], I16,
                              tag=f"ixs{_layer2.qrr[0] % 4}")
                nc.sync.dma_start(
                    ixt[:], t_idx16[:, woff + a * 8:woff + b * 8])
                dma_gather_nopad(nc.gpsimd, out_ap, in_ap, ixt[:],
                                 (b - a) * P, F, TABW,
                                 queue_num=_layer2.qrr[0] % 4)
                _layer2.qrr[0] += 1
            # scores for this quarter block (uniform J within block)
            gv = gath[:].rearrange("p (t j f) -> p t j f", j=Jq[q], f=F)
            s = pb.tile([P, ncols * H], F32, tag=f"s2_{qi}")
            s4 = s[:].rearrange("p (t j h) -> p t j h", j=Jq[q], h=H)
            adv = ad[:][:, t0:t0 + nt, None]
            nc.vector.tensor_tensor(out=s4, in0=gv[:, :, :, CW:CW + H],
                                    in1=adv.to_broadcast([P, nt, Jq[q], H]),
                                    op=ALU.add)
            r = pb.tile([P, ncols * H], F32, tag=f"r2_{qi}")
            nc.scalar.activation(r[:], s[:], AF.Relu)
            u = pb.tile([P, ncols * H], F32, tag=f"u2_{qi}")
            nc.vector.tensor_scalar(out=u[:], in0=r[:], scalar1=4.0, scalar2=None,
                                    op0=ALU.mult)
            nc.vector.tensor_tensor(out=u[:], in0=u[:], in1=s[:], op=ALU.add)
            rhs2 = pb.tile([P, ncols * FV], BF16, tag=f"rhs2b_{qi}")
            r2 = rhs2[:].rearrange("p (t j f) -> p t j f", j=Jq[q], f=FV)
            nc.scalar.activation(r2[:, :, :, 0:H],
                                 u[:].rearrange("p (t j h) -> p t j h", j=Jq[q], h=H),
                                 AF.Exp, scale=0.2)
            nc.vector.tensor_tensor(
                out=r2[:, :, :, H:],
                in0=gv[:, :, :, 0:CW],
                in1=r2[:, :, :, 0:H].to_broadcast([P, nt, Jq[q], H, CW // H]),
                op=ALU.mult)
            rhs2q[q] = rhs2
            woff += ncols * 8
        ost = fin.tile([P, nt * CW], F32, tag="ofin2")
        for i in range(nt):
            t = t0 + i
            acc = accp.tile([P, FV], F32, tag="agg")
            for qi, q in enumerate(nz):
                for j in range(Jq[q]):
                    col = i * Jq[q] + j
                    nc.tensor.matmul(
                        acc[:], lhsT=ident[:],
                        rhs=rhs2q[q][:, col * FV:(col + 1) * FV],
                        start=(qi == 0 and j == 0),
                        stop=(qi == len(nz) - 1 and j == Jq[q] - 1))
            z = fin.tile([P, H], F32, tag="zf2")
            nc.vector.tensor_tensor(out=z[:], in0=acc[:, 0:H],
                                    in1=zc[:, t * H:(t + 1) * H], op=ALU.subtract)
            nc.vector.tensor_scalar(out=z[:], in0=z[:], scalar1=1e-16, scalar2=None,
                                    op0=ALU.add)
            nc.vector.reciprocal(z[:], z[:])
            nc.vector.tensor_tensor(
                out=ost[:, i * CW:(i + 1) * CW].rearrange("p (h c) -> p h c", h=H),
                in0=acc[:, H:].rearrange("p (h c) -> p h c", h=H),
                in1=z[:].to_broadcast([P, H, CW // H]),
                op=ALU.mult)
        _finalize_group(nc, fin, out_res, ost, t0, nt, CW, H, br)


def _make_inputs(cfg, percore, waug1, waug2, inputs):
    x = np.asarray(inputs["x"], np.float32)
    xpad = np.zeros((cfg.NPAD, cfg.FIN), np.float32)
    xpad[:cfg.N] = x
    wf = np.asarray(inputs["Wf"], np.float32).astype(NPBF16)
    b1r = np.broadcast_to(np.asarray(inputs["b1"], np.float32), (P, cfg.CW1)).copy()
    b2r = np.broadcast_to(np.asarray(inputs["b2"], np.float32), (P, cfg.C2)).copy()
    bfr = np.broadcast_to(np.asarray(inputs["bf"], np.float32), (P, cfg.NCOUT)).copy()
    in_maps = []
    for c in range(cfg.ncores):
        pc = percore[c]
        xownT = np.ascontiguousarray(xpad[pc["own"]].T).astype(NPBF16)
        in_maps.append({
            "xexpT": pc["xexpT"], "xownT": xownT,
            "waug1": waug1.astype(NPBF16), "waug2": waug2.astype(NPBF16),
            "wf": wf, "idx16": pc["idx16"],
            "npad1": pc["npad1"], "npad2": pc["npad2"],
            "b1r": b1r, "b2r": b2r, "bfr": bfr,
        })
    return in_maps


def _assemble(cfg, percore, results):
    out = np.zeros((cfg.NPAD, cfg.NCOUT), np.float32)
    for c in range(cfg.ncores):
        out[percore[c]["own"]] = results[c]["out"]
    return out[:cfg.N]


def kernel(**inputs) -> np.ndarray:
    cfg = Cfg(N=100000, E=800000, ncores=8)
    percore, waug1, waug2, meta = preprocess(
        cfg,
        np.asarray(inputs["x"], np.float32),
        np.asarray(inputs["edge_index"]),
        np.asarray(inputs["W1"], np.float32),
        np.asarray(inputs["a_src1"], np.float32),
        np.asarray(inputs["a_dst1"], np.float32),
        np.asarray(inputs["W2"], np.float32),
        np.asarray(inputs["a_src2"], np.float32),
        np.asarray(inputs["a_dst2"], np.float32),
    )
    nc = build_nc(cfg, meta)
    in_maps = _make_inputs(cfg, percore, waug1, waug2, inputs)
    res = run_bass_kernel_spmd(nc, in_maps, core_ids=list(range(cfg.ncores)))
    return _assemble(cfg, percore, res.results)


if __name__ == "__main__":
    import reference as R
    inputs = R.setup_inputs()
    out = kernel(**{k: np.asarray(v) for k, v in inputs.items()})
    print("out", out.shape, out.dtype)
